# revision 1
# baseline (speedup 1.0000x reference)
"""TRN2 Bass kernel for nn_BetweenClusterFC.

Computes out[n] = sum_f (emb_1 @ W1 + b1)[n,f] * (emb_2 @ W2 + b2)[n,f]
for emb_1/emb_2 [32768, 1024] fp32, W [1024, 512], b [512], out [32768] fp32.

Sharding: data-parallel over the 8 NeuronCores — each core handles 4096 rows;
W1/b1/W2/b2 replicated. No cross-core communication; outputs concatenated on
the host.

Numerics/layout strategy:
  - The embeddings are transposed host-side so each core gets eT [1024, 4096]
    with the contraction dim outermost — matmul lhsT tiles [128 D-chunk,
    128 rows] DMA straight from DRAM (contiguous bursts), eliminating all
    on-device transposes.
  - Each fp32 operand X is split host-side into fp16 hi/lo halves
    (Xh = fp16(X), Xl = fp16(X - Xh); the TRN2 PE handles fp16 subnormals
    exactly, verified on HW). The product is evaluated as three full-rate
    fp16 matmuls accumulated in fp32 PSUM:
        X @ W  ~=  Xh@Wh + Xh@Wl + Xl@Wh     (dropped term is O(2^-22))
    A native fp32 matmul costs 4 PE cycles/row on cayman; the 3-pass fp16
    scheme costs 3 with fp32-grade accuracy (measured ~1.1e-6 max rel err
    vs the fp32 reference, comparable to a pure-fp32 kernel's ~9e-7).
  - Per 128-row tile: the two inputs' 24-matmul accumulation groups are
    interleaved per k-chunk into two PSUM banks (more independent work for
    the PE reorder window); DVE adds the bias, multiplies h1*h2 and reduces
    along the free dim into acc[:, tile]; a final PE transpose of acc
    [128, 32] yields a contiguous [32, 128] store of the 4096 outputs.

Startup: W1 + first tiles load ahead of W2 in consumption order; PE warmup
transposes span the startup-DMA window so real matmuls begin at full clock.
Measured on trn2 (8 cores, SPMD): ~363 us HW exec, max rel err ~1.1e-6.
"""

import sys
import time

import numpy as np

if "/opt/trn_rl_repo" not in sys.path:
    sys.path.insert(0, "/opt/trn_rl_repo")

import concourse.mybir as mybir
import concourse.tile as tile
from concourse import bacc
from concourse.bass_utils import run_bass_kernel_spmd
from concourse.masks import make_identity

F32 = mybir.dt.float32
F16 = mybir.dt.float16

N = 32768
D = 1024
F = 512
P = 128
NCORES = 8
R = N // NCORES  # rows per core
RT = R // P      # 128-row tiles per core
KC = D // P      # contraction chunks

_CACHE = {}


def split_f16(x):
    hi = x.astype(np.float16)
    lo = (x - hi.astype(np.float32)).astype(np.float16)
    return hi, lo


def _build_program(rows=R, compile=True):
    rt_count = rows // P
    nc = bacc.Bacc("TRN2", target_bir_lowering=False, debug=False)

    def din(name, shape, dt=F16):
        return nc.dram_tensor(name, shape, dt, kind="ExternalInput").ap()

    e1h = din("e1h", [D, rows])
    e1l = din("e1l", [D, rows])
    e2h = din("e2h", [D, rows])
    e2l = din("e2l", [D, rows])
    w1h = din("w1h", [D, F])
    w1l = din("w1l", [D, F])
    w2h = din("w2h", [D, F])
    w2l = din("w2l", [D, F])
    b1 = din("b1", [F], F32)
    b2 = din("b2", [F], F32)
    out = nc.dram_tensor("out", [rows], F32, kind="ExternalOutput").ap()

    mult = mybir.AluOpType.mult
    add = mybir.AluOpType.add

    r3 = lambda ap: ap.rearrange("(kc p) r -> p kc r", p=P)
    e1h3, e1l3, e2h3, e2l3 = r3(e1h), r3(e1l), r3(e2h), r3(e2l)

    with tile.TileContext(nc) as tc:
        with (
            tc.tile_pool(name="consts", bufs=1) as consts,
            tc.tile_pool(name="etpool", bufs=3) as etpool,
            tc.tile_pool(name="hpool", bufs=2) as hpool,
            tc.tile_pool(name="fin", bufs=1) as fin_pool,
            tc.tile_pool(name="tp_psum", bufs=1, space="PSUM") as tp_psum,
            tc.tile_pool(name="h_psum", bufs=3, space="PSUM") as h_psum,
        ):
            ident = consts.tile([P, P], F32)
            make_identity(nc, ident)

            w1h_sb = consts.tile([P, KC, F], F16, tag="w1h")
            nc.sync.dma_start(w1h_sb[:], w1h.rearrange("(kc p) f -> p kc f", p=P))
            w1l_sb = consts.tile([P, KC, F], F16, tag="w1l")
            nc.sync.dma_start(w1l_sb[:], w1l.rearrange("(kc p) f -> p kc f", p=P))
            w2h_sb = consts.tile([P, KC, F], F16, tag="w2h")
            w2l_sb = consts.tile([P, KC, F], F16, tag="w2l")

            b1_bc = consts.tile([P, F], F32, tag="b1")
            nc.gpsimd.dma_start(b1_bc[:], b1[None, :].to_broadcast((P, F)))
            b2_bc = consts.tile([P, F], F32, tag="b2")
            nc.gpsimd.dma_start(b2_bc[:], b2[None, :].to_broadcast((P, F)))

            # warm the PE across the whole startup-DMA window so the first
            # real matmuls run at full clock (HAM re-throttles after ~3.4us idle)
            warm_rhs = ident[:, None, :].to_broadcast((P, 4, P))
            warm_ps = h_psum.tile([P, F], F32, tag="h0")
            for _ in range(22):
                nc.tensor.transpose(warm_ps[:], ident[:], warm_rhs)

            acc = fin_pool.tile([P, rt_count], F32, tag="acc")

            for rt in range(rt_count):
                ets, hps = [], []
                for j, (eh3, el3) in enumerate(((e1h3, e1l3), (e2h3, e2l3))):
                    eth = etpool.tile([P, KC, P], F16, tag=f"eth{j}")
                    nc.sync.dma_start(eth[:], eh3[:, :, rt * P:(rt + 1) * P])
                    etl = etpool.tile([P, KC, P], F16, tag=f"etl{j}")
                    nc.sync.dma_start(etl[:], el3[:, :, rt * P:(rt + 1) * P])
                    if rt == 0 and j == 0:
                        nc.sync.dma_start(
                            w2h_sb[:], w2h.rearrange("(kc p) f -> p kc f", p=P))
                        nc.sync.dma_start(
                            w2l_sb[:], w2l.rearrange("(kc p) f -> p kc f", p=P))
                    ets.append((eth, etl))
                    hps.append(h_psum.tile([P, F], F32, tag=f"h{j}", name=f"hp{j}"))

                ws = ((w1h_sb, w1l_sb), (w2h_sb, w2l_sb))
                for kc in range(KC):
                    for j in range(2):
                        (eth, etl), (wh_sb, wl_sb) = ets[j], ws[j]
                        for pi, (lhs, rhs) in enumerate((
                            (eth[:, kc, :], wh_sb[:, kc, :]),
                            (eth[:, kc, :], wl_sb[:, kc, :]),
                            (etl[:, kc, :], wh_sb[:, kc, :]),
                        )):
                            nc.tensor.matmul(
                                hps[j][:], lhsT=lhs, rhs=rhs,
                                start=(kc == 0 and pi == 0),
                                stop=(kc == KC - 1 and pi == 2),
                            )

                hts = []
                for j, b_bc in enumerate((b1_bc, b2_bc)):
                    ht = hpool.tile([P, F], F32, tag=f"ht{j}")
                    nc.vector.tensor_tensor(ht[:], hps[j][:], b_bc[:], add)
                    hts.append(ht)

                prod = hpool.tile([P, F], F32, tag="prod")
                nc.vector.tensor_tensor(prod[:], hts[0][:], hts[1][:], mult)
                nc.vector.tensor_reduce(
                    acc[:, rt:rt + 1], prod[:],
                    axis=mybir.AxisListType.X, op=add,
                )

            # acc [128 rows-in-tile, rt_count tiles] -> out[rt*128 + p]
            ps_fin = tp_psum.tile([rt_count, P], F32, tag="tp")
            nc.tensor.transpose(ps_fin[:], acc[:], ident[:])
            fin = fin_pool.tile([rt_count, P], F32, tag="fin_sb")
            nc.vector.tensor_copy(fin[:], ps_fin[:])
            nc.sync.dma_start(out.rearrange("(rt p) -> rt p", p=P), fin[:])

    if compile:
        nc.compile()
    return nc


def _get_program():
    if "nc" not in _CACHE:
        _CACHE["nc"] = _build_program()
    return _CACHE["nc"]


def make_in_maps(emb_1, emb_2, W1, b1, W2, b2):
    e1t = np.ascontiguousarray(np.asarray(emb_1, dtype=np.float32).T)
    e2t = np.ascontiguousarray(np.asarray(emb_2, dtype=np.float32).T)
    e1h, e1l = split_f16(e1t)
    e2h, e2l = split_f16(e2t)
    w1h, w1l = split_f16(np.ascontiguousarray(np.asarray(W1, dtype=np.float32)))
    w2h, w2l = split_f16(np.ascontiguousarray(np.asarray(W2, dtype=np.float32)))
    b1 = np.ascontiguousarray(np.asarray(b1, dtype=np.float32))
    b2 = np.ascontiguousarray(np.asarray(b2, dtype=np.float32))
    return [
        {
            "e1h": e1h[:, c * R:(c + 1) * R], "e1l": e1l[:, c * R:(c + 1) * R],
            "e2h": e2h[:, c * R:(c + 1) * R], "e2l": e2l[:, c * R:(c + 1) * R],
            "w1h": w1h, "w1l": w1l, "w2h": w2h, "w2l": w2l,
            "b1": b1, "b2": b2,
        }
        for c in range(NCORES)
    ]


def kernel(emb_1, emb_2, W1, b1, W2, b2, **_unused):
    nc = _get_program()
    in_maps = make_in_maps(emb_1, emb_2, W1, b1, W2, b2)
    last_err = None
    for attempt in range(3):
        try:
            res = run_bass_kernel_spmd(nc, in_maps, list(range(NCORES))).results
            return np.concatenate([res[c]["out"] for c in range(NCORES)])
        except Exception as e:  # transient NRT/axon failures observed; retry
            last_err = e
            time.sleep(2.0 * (attempt + 1))
    raise last_err



# revision 4
# speedup vs baseline: 2.6942x; 2.6942x over previous
"""TRN2 Bass kernel for nn_BetweenClusterFC (v2: single-pass fp16).

Computes out[n] = sum_f (emb_1 @ W1 + b1)[n,f] * (emb_2 @ W2 + b2)[n,f]
for emb_1/emb_2 [32768, 1024] fp32, W [1024, 512], b [512], out [32768] fp32.

Sharding: data-parallel over the 8 NeuronCores — each core handles 4096 rows;
W1/W2 replicated. No cross-core communication; outputs concatenated on the
host.

Strategy (v2):
  - The correctness gate is rel_err < 2e-2; a single fp16 pass per operand
    measures 3.4e-4 on the real inputs, so the v1 3-pass hi/lo fp16 scheme
    (1e-6) wastes 3x PE cycles. One fp16 matmul pass per 128-deep k-chunk is
    the sweet spot: fp8(e4m3) single-pass fails the gate (4.8e-2) and the
    full-precision fp8 DoubleRow variants are not faster than fp16 on HW
    (DR matmul +13%, LDWEIGHTS +72% per the TRN2 engine docs).
  - Embeddings are transposed host-side to eT [1024, 4096] fp16 so matmul
    lhsT tiles [128 D-chunk, 128 rows] DMA straight from DRAM; e-tiles are
    fetched 2 row-tiles at a time ([128, 8, 256], 512B bursts).
  - Biases are folded out of the device program algebraically:
        out = rowsum(h1*h2) + E1@(W1 b2) + E2@(W2 b1) + b1.b2
    The rank-1 corrections are applied host-side (they are exactly zero for
    this problem's inputs, so the runtime path is a no-op check).
  - Per 128-row tile: the two inputs' 8-matmul accumulation groups are
    interleaved per k-chunk into two PSUM banks; DVE multiplies h1*h2 and
    reduces along the free dim into acc[:, tile]; a final PE transpose of
    acc [128, 32] yields a contiguous [32, 128] store of the 4096 outputs.
  - PE warmup transposes span the startup-DMA window so real matmuls begin
    at full clock.
"""

import sys
import time

import numpy as np

if "/opt/trn_rl_repo" not in sys.path:
    sys.path.insert(0, "/opt/trn_rl_repo")

import concourse.mybir as mybir
import concourse.tile as tile
from concourse import bacc
from concourse.bass_utils import run_bass_kernel_spmd
from concourse.masks import make_identity

F32 = mybir.dt.float32
F16 = mybir.dt.float16

N = 32768
D = 1024
F = 512
P = 128
NCORES = 8
R = N // NCORES  # rows per core
RT = R // P      # 128-row tiles per core
KC = D // P      # contraction chunks
TW = 2 * P       # e-tile width (2 row-tiles per DMA)

_CACHE = {}


def _build_program(rows=R, compile=True):
    rt_count = rows // P
    nc = bacc.Bacc("TRN2", target_bir_lowering=False, debug=False)

    def din(name, shape, dt=F16):
        return nc.dram_tensor(name, shape, dt, kind="ExternalInput").ap()

    e1h = din("e1h", [D, rows])
    e2h = din("e2h", [D, rows])
    w1h = din("w1h", [D, F])
    w2h = din("w2h", [D, F])
    out = nc.dram_tensor("out", [rows], F32, kind="ExternalOutput").ap()

    mult = mybir.AluOpType.mult
    add = mybir.AluOpType.add

    r3 = lambda ap: ap.rearrange("(kc p) r -> p kc r", p=P)
    e1h3, e2h3 = r3(e1h), r3(e2h)

    with tile.TileContext(nc) as tc:
        with (
            tc.tile_pool(name="consts", bufs=1) as consts,
            tc.tile_pool(name="etpool", bufs=3) as etpool,
            tc.tile_pool(name="hpool", bufs=2) as hpool,
            tc.tile_pool(name="fin", bufs=1) as fin_pool,
            tc.tile_pool(name="tp_psum", bufs=1, space="PSUM") as tp_psum,
            tc.tile_pool(name="h_psum", bufs=3, space="PSUM") as h_psum,
        ):
            ident = consts.tile([P, P], F32)
            make_identity(nc, ident)

            w1h_sb = consts.tile([P, KC, F], F16, tag="w1h")
            nc.sync.dma_start(w1h_sb[:], w1h.rearrange("(kc p) f -> p kc f", p=P))
            w2h_sb = consts.tile([P, KC, F], F16, tag="w2h")
            nc.sync.dma_start(w2h_sb[:], w2h.rearrange("(kc p) f -> p kc f", p=P))

            # warm the PE across the whole startup-DMA window so the first
            # real matmuls run at full clock (HAM re-throttles after ~3.4us idle)
            warm_rhs = ident[:, None, :].to_broadcast((P, 4, P))
            warm_ps = h_psum.tile([P, F], F32, tag="h0")
            for _ in range(14):
                nc.tensor.transpose(warm_ps[:], ident[:], warm_rhs)

            acc = fin_pool.tile([P, rt_count], F32, tag="acc")

            ets = [None, None]
            for rt in range(rt_count):
                col = (rt % 2) * P
                if rt % 2 == 0:
                    tp = rt // 2
                    for j, eh3 in enumerate((e1h3, e2h3)):
                        eth = etpool.tile([P, KC, TW], F16, tag=f"eth{j}")
                        nc.sync.dma_start(eth[:], eh3[:, :, tp * TW:(tp + 1) * TW])
                        ets[j] = eth

                hps = [
                    h_psum.tile([P, F], F32, tag=f"h{j}", name=f"hp{j}")
                    for j in range(2)
                ]
                ws = (w1h_sb, w2h_sb)
                for kc in range(KC):
                    for j in range(2):
                        nc.tensor.matmul(
                            hps[j][:],
                            lhsT=ets[j][:, kc, col:col + P],
                            rhs=ws[j][:, kc, :],
                            start=(kc == 0),
                            stop=(kc == KC - 1),
                        )

                # DVE can read only one PSUM operand per instruction: stage h1
                # through SBUF on the (otherwise idle) ACT engine, then fuse
                # multiply + free-dim reduce in a single DVE op.
                h1sb = hpool.tile([P, F], F32, tag="h1sb")
                nc.scalar.copy(h1sb[:], hps[0][:])
                prod = hpool.tile([P, F], F32, tag="prod")
                nc.vector.tensor_tensor(prod[:], hps[1][:], h1sb[:], mult)
                nc.vector.tensor_reduce(
                    acc[:, rt:rt + 1], prod[:],
                    axis=mybir.AxisListType.X, op=add,
                )

            # acc [128 rows-in-tile, rt_count tiles] -> out[rt*128 + p]
            ps_fin = tp_psum.tile([rt_count, P], F32, tag="tp")
            nc.tensor.transpose(ps_fin[:], acc[:], ident[:])
            fin = fin_pool.tile([rt_count, P], F32, tag="fin_sb")
            nc.vector.tensor_copy(fin[:], ps_fin[:])
            nc.sync.dma_start(out.rearrange("(rt p) -> rt p", p=P), fin[:])

    if compile:
        nc.compile()
    return nc


def _get_program():
    if "nc" not in _CACHE:
        _CACHE["nc"] = _build_program()
    return _CACHE["nc"]


def make_in_maps(emb_1, emb_2, W1, b1, W2, b2):
    e1t = np.ascontiguousarray(
        np.asarray(emb_1, dtype=np.float32).astype(np.float16).T)
    e2t = np.ascontiguousarray(
        np.asarray(emb_2, dtype=np.float32).astype(np.float16).T)
    w1h = np.asarray(W1, dtype=np.float32).astype(np.float16)
    w2h = np.asarray(W2, dtype=np.float32).astype(np.float16)
    return [
        {
            "e1h": e1t[:, c * R:(c + 1) * R],
            "e2h": e2t[:, c * R:(c + 1) * R],
            "w1h": w1h, "w2h": w2h,
        }
        for c in range(NCORES)
    ]


def kernel(emb_1, emb_2, W1, b1, W2, b2, **_unused):
    nc = _get_program()
    in_maps = make_in_maps(emb_1, emb_2, W1, b1, W2, b2)
    last_err = None
    for attempt in range(3):
        try:
            res = run_bass_kernel_spmd(nc, in_maps, list(range(NCORES))).results
            out = np.concatenate([res[c]["out"] for c in range(NCORES)])
            break
        except Exception as e:  # transient NRT/axon failures observed; retry
            last_err = e
            time.sleep(2.0 * (attempt + 1))
    else:
        raise last_err

    # bias terms, folded out of the device program:
    # out += E1 @ (W1 b2) + E2 @ (W2 b1) + b1.b2  (all zero for this problem)
    b1 = np.asarray(b1, dtype=np.float32)
    b2 = np.asarray(b2, dtype=np.float32)
    if b1.any() or b2.any():
        W1 = np.asarray(W1, dtype=np.float32)
        W2 = np.asarray(W2, dtype=np.float32)
        e1 = np.asarray(emb_1, dtype=np.float32)
        e2 = np.asarray(emb_2, dtype=np.float32)
        out = out + e1 @ (W1 @ b2) + e2 @ (W2 @ b1) + float(b1 @ b2)
    return out
